# revision 16
# baseline (speedup 1.0000x reference)
"""Trainium2 Bass kernel for nn_DirectionVarEntropy.

Computes, per 14x14 patch and channel:
  - pixel-value entropy (256-bin histogram of round(x*255))
  - direction variance psi of 3x3-DCT sliding-window directional stds
  - richness = mean_c(psi_m * entropy)  ->  output (B, Hp, Wp)

Sharding: pure data parallel over batch, 2 images per core on 8 cores.

Per-core layout: 2048 spatial patches x 3 channels = 6144 patch-channels,
mapped to [128 partitions x 48 free segments]; seg s = t*3 + c where
t = spatial_patch // 128, partition p = spatial_patch % 128.

Entropy: sort each seg's 196 pixel codes (bitonic merge network, 36
min/max stages in bf16 at the DVE 2x rate), then per-pixel own-bin
counts from run lengths via custom DVE scans; E = log2(196) -
mean_p ln(c_p)/ln 2.  Sort stages are interleaved into the per-seg psi
pipeline as DVE filler.

Psi (new): per direction group g the 3 DCT values' variance is written
with two orthogonal contrasts d1,d2 (linear in the window pixels):
sigma_g^2 * 3 = d1^2 + d2^2.  All 16 contrast planes for the 144
windows of a (seg, wj) column come from ONE PE matmul with the patch
data as the stationary operand ([126pix x 128patch] lhsT) and a
wj-shifted constant weight table as the moving operand ([126 x 192]).
Squares: even contrasts via ACT Square (PSUM->SBUF, fused drain), odd
via a custom DVE op out = Src0^2 + Src1 (one PSUM input allowed).
sigma via ACT Sqrt; S1..S4/A/e-contrasts as strided bf16 adds; psi =
(e1^2/2 + e2^2/2 + e3^2/4) / (3 (A/4 + 1e-8)^2).
"""

import functools
import math

import numpy as np

import concourse.bacc as bacc
import concourse.bass as bass
import concourse.mybir as mybir
from concourse import bass_utils
from concourse.tile import TileContext
from concourse.masks import make_identity

# ---------------- custom DVE ops (registered at import) ----------------
import concourse.dve_ops as dve_ops
from concourse.dve_spec import (Spec, Src0, Src1, C0, C1, Zero, MaxNeg,
                                eq, maxx, select, scan, lower as dve_lower,
                                AluOp, Idx, _has_src1)
from concourse.dve_uop import DveOpSpec
from concourse.bass import BassVectorEngine


def _register(name: str, spec: Spec, subdim: bool = False):
    for op in dve_ops.OPS:
        if op.name == name:
            return op
    row = dve_ops._CUSTOM_DVE_ROW_BASE + len(dve_ops.OPS)
    assert row < 0x20, "custom DVE op rows exhausted"
    shas = {}
    for ver in ("v3", "v4"):
        s = DveOpSpec(name=name, opcode=row, uops=dve_lower(spec, ver=ver),
                      rd1_en=_has_src1(spec))
        shas[ver] = s.sha(ver)
    op = dve_ops.DveOp(name, spec, subdim=subdim, uops_sha=shas)
    dve_ops.OPS.append(op)
    dve_ops._SUB_OPCODE_FOR_NAME[name] = row
    dve_ops.CUSTOM_DVE_SPECS[name] = spec
    return op


def _np_bidx(fill):
    def ref(in0, in1, s0, s1, imm2):
        n = int(np.prod(in0.shape[1:]))
        idx = np.arange(n, dtype=np.float32).reshape((1,) + in0.shape[1:])
        return np.where(in0 == in1, np.float32(fill), idx).astype(np.float32)
    return ref


_BIDX_NEG = _register(
    "ATH_BIDX_NEG",
    Spec(body=select(eq(Src0, Src1), MaxNeg, Idx),
         reference=_np_bidx(-3.4028235e38)))
_BIDX_POS = _register(
    "ATH_BIDX_POS",
    Spec(body=select(eq(Src0, Src1), Zero - MaxNeg, Idx),
         reference=_np_bidx(3.4028235e38)))
_SCAN_MAX = _register(
    "ATH_SCAN_MAX",
    Spec(body=scan(AluOp.MAX, Src0),
         reference=lambda in0, in1, s0, s1, imm2:
         np.maximum.accumulate(
             in0.reshape(in0.shape[0], -1), axis=1).reshape(in0.shape)))
_SCAN_MIN = _register(
    "ATH_SCAN_MIN",
    Spec(body=scan(AluOp.MIN, Src0, init=C0),
         reference=lambda in0, in1, s0, s1, imm2:
         np.minimum.accumulate(
             np.minimum(in0, s0).reshape(in0.shape[0], -1),
             axis=1).reshape(in0.shape)))
_CONV2 = _register(
    "ATH_CONV2",
    Spec(body=Src0 * C0 + Src1 * C1,
         reference=lambda in0, in1, s0, s1, imm2: in0 * s0 + in1 * s1))
_SQADD = _register(
    "ATH_SQADD",
    Spec(body=Src0 * Src0 + Src1,
         reference=lambda in0, in1, s0, s1, imm2: in0 * in0 + in1))


def _v_bidx_neg(self, out, in0, in1):
    return self._custom_dve(_BIDX_NEG, out=out, in0=in0, in1=in1)


def _v_bidx_pos(self, out, in0, in1):
    return self._custom_dve(_BIDX_POS, out=out, in0=in0, in1=in1)


def _v_scan_max(self, out, in0):
    return self._custom_dve(_SCAN_MAX, out=out, in0=in0)


def _v_scan_min(self, out, in0, init):
    return self._custom_dve(_SCAN_MIN, out=out, in0=in0, s0=init)


def _v_conv2(self, out, in0, in1, c0, c1):
    return self._custom_dve(_CONV2, out=out, in0=in0, in1=in1, s0=c0, s1=c1)


def _v_sqadd(self, out, in0, in1):
    return self._custom_dve(_SQADD, out=out, in0=in0, in1=in1)


BassVectorEngine.ath_bidx_neg = _v_bidx_neg
BassVectorEngine.ath_bidx_pos = _v_bidx_pos
BassVectorEngine.ath_scan_max = _v_scan_max
BassVectorEngine.ath_scan_min = _v_scan_min
BassVectorEngine.ath_conv2 = _v_conv2
BassVectorEngine.ath_sqadd = _v_sqadd

# ---------------- problem constants ----------------
P = 128
PH = 14
NWIN = 12          # sliding 3x3 positions per axis
NPIX = PH * PH     # 196
LN2 = 0.6931471805599453
F32 = mybir.dt.float32
BF16 = mybir.dt.bfloat16
ALU = mybir.AluOpType
ACTF = mybir.ActivationFunctionType

B_FULL, C, H, W = 16, 3, 448, 448
N_CORES = 8
B_CORE = B_FULL // N_CORES      # 2
HP = H // PH                    # 32
T_BLKS = B_CORE * HP * HP // P  # 16 t-blocks of 128 spatial patches
SEGS = T_BLKS * C               # 48

SW = 256                        # sort width per seg (padded)
PADV = 320.0                    # pad value > max code 255
SDOM = NPIX + 1                 # scan domain per seg (196 codes + 1 pad)
BIG = 3.0e38

GROUPS = ([[(r, 0), (r, 1), (r, 2)] for r in range(3)]
          + [[(0, c), (1, c), (2, c)] for c in range(3)]
          + [[(0, 0), (1, 1), (2, 2)], [(0, 2), (1, 1), (2, 0)]])
C1V = np.array([1.0, -1.0, 0.0]) / np.sqrt(2.0)
C2V = np.array([1.0, 1.0, -2.0]) / np.sqrt(6.0)


def build_wsh(dct_flat):
    """Shifted contrast weight tables (f32).
    W0 [126, 6*192] for wj 0..5 against XT tile T0 (pix rows 0..125),
    W1 [126, 7*192] for wj 5..11 against T1 (pix rows 70..195).
    Feature index f = g*24 + wi*2 + c2; pixel row = dj*14 + i."""
    D = np.asarray(dct_flat, np.float64).reshape(3, 3)
    W = np.zeros((3, PH, 8, NWIN, 2), np.float64)
    for g, mem in enumerate(GROUPS):
        for c2i, cvec in enumerate((C1V, C2V)):
            for mi, (r, c) in enumerate(mem):
                for di in range(3):
                    for wi in range(NWIN):
                        for dj in range(3):
                            W[dj, wi + di, g, wi, c2i] += (
                                cvec[mi] * D[r, di] * D[c, dj])
    W42 = W.reshape(42, 192)
    W0 = np.zeros((126, 6, 192), np.float64)
    for wj in range(6):
        W0[14 * wj:14 * wj + 42, wj] = W42
    W1 = np.zeros((126, 7, 192), np.float64)
    for wj in range(5, 12):
        base = 14 * (wj - 5)
        W1[base:base + 42, wj - 5] = W42
    return (W0.reshape(126, -1).astype(np.float32),
            W1.reshape(126, -1).astype(np.float32))


def _sort_stages(nc, SRT, Bp):
    """Generator yielding one bitonic stage (2 DVE instrs) per next().
    See baseline docstring: pads at the end are global maxima; trailing
    all-pad merge blocks are skipped and must be pre-set in BOTH buffers."""
    ping = lambda: SRT[:, :, 1:1 + SW]
    pong = lambda: Bp[:, :, :]
    cur_in, cur_out = ping, pong

    def cmpex(lo_out, hi_out, lo_a, lo_b, hi_a, hi_b):
        nc.vector.tensor_tensor(lo_out, lo_a, lo_b, ALU.min)
        nc.vector.tensor_tensor(hi_out, hi_a, hi_b, ALU.max)

    nphase = SW.bit_length() - 1          # 8
    for j in range(nphase):
        m = 2 << j
        h = m // 2
        na = (NPIX + m - 1) // m          # active merge blocks
        nfull = NPIX // m                 # blocks fully below the boundary
        ain, aout = cur_in(), cur_out()
        i4 = ain.rearrange("p s (nb m) -> p s nb m", m=m)[:, :, 0:na]
        o4 = aout.rearrange("p s (nb m) -> p s nb m", m=m)[:, :, 0:na]

        def tri(blo, bhi, kmin, kmax):
            ii = i4[:, :, blo:bhi]
            oo = o4[:, :, blo:bhi]
            if h > 1:
                hr = ii[:, :, :, m - 1:h - 1:-1]
                lr = ii[:, :, :, h - 1::-1]
            else:
                hr = ii[:, :, :, m - 1:m]
                lr = ii[:, :, :, 0:1]
            if kmin > 0:
                nc.vector.tensor_tensor(
                    oo[:, :, :, 0:kmin], ii[:, :, :, 0:kmin],
                    hr[:, :, :, 0:kmin], ALU.min)
            if kmax > 0:
                nc.vector.tensor_tensor(
                    oo[:, :, :, h:h + kmax], ii[:, :, :, h:h + kmax],
                    lr[:, :, :, 0:kmax], ALU.max)

        if nfull > 0:
            tri(0, nfull, h, h)
        if na > nfull:                    # boundary block, partial lanes
            base = nfull * m
            tri(nfull, na, min(h, NPIX - base),
                max(0, NPIX - base - h))
        cur_in, cur_out = cur_out, cur_in
        yield
        d = h // 2
        while d >= 1:
            nc_act = (NPIX + 2 * d - 1) // (2 * d)
            ain, aout = cur_in(), cur_out()
            i4 = ain.rearrange("p s (nb t) -> p s nb t",
                               t=2 * d)[:, :, 0:nc_act]
            o4 = aout.rearrange("p s (nb t) -> p s nb t",
                                t=2 * d)[:, :, 0:nc_act]
            cmpex(o4[:, :, :, 0:d], o4[:, :, :, d:2 * d],
                  i4[:, :, :, 0:d], i4[:, :, :, d:2 * d],
                  i4[:, :, :, 0:d], i4[:, :, :, d:2 * d])
            cur_in, cur_out = cur_out, cur_in
            yield
            d //= 2
    assert cur_in == ping


def _build(dct_flat: tuple) -> bass.Bass:
    nc = bacc.Bacc("TRN2", debug=False, enable_asserts=False)

    x_d = nc.dram_tensor("x", (B_CORE, C, H, W), F32, kind="ExternalInput")
    w0_d = nc.dram_tensor("w0", (126, 6 * 192), F32, kind="ExternalInput")
    w1_d = nc.dram_tensor("w1", (126, 7 * 192), F32, kind="ExternalInput")
    out_d = nc.dram_tensor("out", (B_CORE, HP, HP), F32, kind="ExternalOutput")
    xv = x_d.ap().rearrange("b c (hp i) (wp j) -> b c hp wp i j", i=PH, j=PH)
    ov = out_d.ap()

    segs = SEGS

    with TileContext(nc) as tc:
        with tc.tile_pool(name="persist", bufs=1) as pp:
            lnd = pp.tile([P, NPIX], F32)
            CNT = pp.tile([P, segs, NPIX], BF16)
            psi_acc = pp.tile([P, segs], F32)
            e_acc = pp.tile([P, segs], F32)
            rich = pp.tile([P, segs], F32)
            rich3 = rich.rearrange("p (t c) -> p t c", c=C)
            tsum = pp.tile([P, segs // C], F32)
            osb = pp.tile([P, segs // C], F32)
            ident = pp.tile([P, P], BF16)
            WT0 = pp.tile([126, 6 * 192], F32)
            WT1 = pp.tile([126, 7 * 192], F32)
            WB0 = pp.tile([126, 6 * 192], BF16)
            WB1 = pp.tile([126, 7 * 192], BF16)
            T0 = pp.tile([126, segs * P], BF16)
            T1 = pp.tile([126, segs * P], BF16)
            T0v = T0.rearrange("q (s p) -> q s p", p=P)
            T1v = T1.rearrange("q (s p) -> q s p", p=P)
            EPS = pp.tile([P, 1], F32)
            b23n = pp.tile([P, 1], F32)
            SRT = pp.tile([P, segs, 1 + SW], BF16)

            xq_ctx = tc.tile_pool(name="xq", bufs=1)
            xq = xq_ctx.__enter__()
            X = xq.tile([P, segs, PH, PH], F32)
            Xf = X.rearrange("p s i j -> p (s i j)")
            TMP = xq.tile([P, (segs // 8) * NPIX], F32)

            def issue_seg_dmas(s):
                t, c = s // C, s % C
                b = t // (T_BLKS // B_CORE)
                hp0 = (t % (T_BLKS // B_CORE)) * 4
                engs = (nc.sync, nc.sync, nc.sync, nc.sync)
                for p1 in range(4):
                    engs[p1].dma_start(
                        X[p1 * 32:(p1 + 1) * 32, s],
                        xv[b, c, hp0 + p1],
                    )

            nc.sync.dma_start(WT0, w0_d.ap())
            nc.sync.dma_start(WT1, w1_d.ap())
            make_identity(nc, ident)
            nc.vector.memset(EPS, 1e-8)
            nc.vector.memset(b23n, -float(2 ** 23))
            with nc.allow_low_precision(reason="bf16 matmul weights"):
                nc.scalar.copy(WB0, WT0)
                nc.scalar.copy(WB1, WT1)


            # ---- Xbji (bf16, j-major) + XT via PE transposes ----
            xb_ctx = tc.tile_pool(name="xb", bufs=1)
            xb = xb_ctx.__enter__()
            Xbji = xb.tile([P, segs, PH, PH], BF16)   # [p, s, j, i]
            Xjf = Xbji.rearrange("p s j i -> p s (j i)")

            tp_ctx = tc.tile_pool(name="tp", bufs=2, space="PSUM")
            tpp = tp_ctx.__enter__()
            NS8 = 8
            for s8 in range(segs // NS8):
                g0 = s8 * NS8
                for k in range(NS8):
                    issue_seg_dmas(g0 + k)
                # absorber copies (compress the DMA queue sem waits)
                for k in range(NS8):
                    nc.vector.tensor_copy(X[:, g0 + k], X[:, g0 + k])
                with nc.allow_low_precision(reason="bf16 matmul input"):
                    nc.scalar.copy(
                        Xbji[:, g0:g0 + NS8],
                        X[:, g0:g0 + NS8].rearrange("p s i j -> p s j i"))
                TA = tpp.tile([96, NS8, P], BF16, tag="TA", name="TA")
                TB = tpp.tile([32, NS8, P], BF16, tag="TB", name="TB")
                TC = tpp.tile([96, NS8, P], BF16, tag="TC", name="TC")
                TD = tpp.tile([32, NS8, P], BF16, tag="TD", name="TD")
                for k in range(NS8):
                    s = s8 * NS8 + k
                    nc.tensor.transpose(TA[:, k], Xjf[:, s, 0:96], ident)
                    nc.tensor.transpose(TB[0:30, k], Xjf[:, s, 96:126],
                                        ident)
                    nc.tensor.transpose(TC[:, k], Xjf[:, s, 70:166], ident)
                    nc.tensor.transpose(TD[0:30, k], Xjf[:, s, 166:196],
                                        ident)
                sl = slice(s8 * NS8 * P, (s8 * NS8 + NS8) * P)
                with nc.allow_low_precision(reason="bf16 XT"):
                    nc.vector.tensor_copy(
                        T0[0:96, sl], TA.rearrange("q k p -> q (k p)"))
                    nc.scalar.copy(
                        T0[96:126, sl],
                        TB[0:30].rearrange("q k p -> q (k p)"))
                    nc.vector.tensor_copy(
                        T1[0:96, sl], TC.rearrange("q k p -> q (k p)"))
                    nc.scalar.copy(
                        T1[96:126, sl],
                        TD[0:30].rearrange("q k p -> q (k p)"))
            tp_ctx.__exit__(None, None, None)
            xb_ctx.__exit__(None, None, None)

            # ---- quantize codes into SRT (ACT + gpsimd, via 2^23 RNE) ----
            TWO23 = float(2 ** 23)
            qch = (segs // 8) * NPIX
            TMP3 = TMP.rearrange("p (s k) -> p s k", k=NPIX)
            spq = segs // 8
            for q in range(8):
                nc.gpsimd.tensor_scalar(
                    TMP, Xf[:, q * qch:(q + 1) * qch], 255.0, TWO23,
                    ALU.mult, ALU.add)
                nc.scalar.activation(
                    SRT[:, q * spq:(q + 1) * spq, 1:1 + NPIX], TMP3,
                    ACTF.Identity, bias=b23n)
            nc.vector.memset(SRT[:, :, 0:1], -1.0)
            nc.vector.memset(SRT[:, :, 1 + NPIX:1 + SW], PADV)
            xq_ctx.__exit__(None, None, None)

            # ============ psi pipeline with interleaved sort ============
            wp_ctx = tc.tile_pool(name="work", bufs=2)
            wp = wp_ctx.__enter__()
            mm_ctx = tc.tile_pool(name="mm", bufs=2, space="PSUM")
            mmp = mm_ctx.__enter__()
            Bp = wp.tile([P, segs, SW], BF16, tag="BP", name="BP", bufs=1)
            nc.gpsimd.memset(Bp[:, :, NPIX:SW], PADV)
            sorter = _sort_stages(nc, SRT, Bp)
            sort_left = 36

            def emit_sort(k):
                nonlocal sort_left
                for _ in range(min(k, sort_left)):
                    next(sorter)
                    sort_left -= 1

            cs = segs // 8

            def emit_scan_chunk(h):
                s0c = h * cs
                src0 = SRT[:, s0c:s0c + cs, 1:1 + SDOM]
                src1 = SRT[:, s0c:s0c + cs, 0:SDOM]
                SC = wp.tile([P, cs * SDOM], F32, tag="SCS", name="SCS",
                             bufs=2)
                MC = wp.tile([P, cs * SDOM], F32, tag="SCM", name="SCM",
                             bufs=1)
                NX = wp.tile([P, cs * SDOM], F32, tag="SCN", name="SCN",
                             bufs=1)
                nc.vector.ath_bidx_neg(MC, src0, src1)
                nc.vector.ath_scan_max(SC, MC)
                nc.vector.ath_bidx_pos(MC, src0, src1)
                nmax = cs * SDOM
                nc.vector.ath_scan_min(NX[:, nmax - 1::-1],
                                       MC[:, nmax - 1::-1], BIG)
                Svc = SC.rearrange("p (s k) -> p s k", k=SDOM)
                NXv = NX.rearrange("p (s k) -> p s k", k=SDOM)
                with nc.allow_low_precision(reason="counts <= 196 exact"):
                    nc.gpsimd.tensor_tensor(
                        CNT[:, s0c:s0c + cs], NXv[:, :, 1:1 + NPIX],
                        Svc[:, :, 0:NPIX], ALU.subtract)
                for i in range(cs):
                    nc.scalar.activation(
                        lnd, CNT[:, s0c + i], ACTF.Ln,
                        accum_out=e_acc[:, s0c + i:s0c + i + 1])

            WB0v = WB0.rearrange("q (w f) -> q w f", f=192)
            WB1v = WB1.rearrange("q (w f) -> q w f", f=192)
            NB = 4   # segs per stats batch

            def emit_stats(sb):
                """S/A/e + psi for a 4-seg batch whose sigma is in SG3."""
                SG3, E4, s0 = sb
                # views [p, b, g, w] with w = 144
                g = lambda i: SG3[:, :, i]
                e1, e2, e3, A = (E4[:, :, 0], E4[:, :, 1], E4[:, :, 2],
                                 E4[:, :, 3])
                S1, S2, t1, t2 = g(0), g(3), g(1), g(6)
                with nc.allow_low_precision(reason="bf16 stats"):
                    nc.vector.ath_conv2(S1, g(0), g(1), 1 / 3, 1 / 3)
                    nc.vector.scalar_tensor_tensor(
                        S1, g(2), 1 / 3, S1, ALU.mult, ALU.add)
                    nc.vector.ath_conv2(S2, g(3), g(4), 1 / 3, 1 / 3)
                    nc.vector.scalar_tensor_tensor(
                        S2, g(5), 1 / 3, S2, ALU.mult, ALU.add)
                    nc.vector.tensor_tensor(e1, S1, S2, ALU.subtract)
                    nc.vector.tensor_tensor(e2, g(6), g(7), ALU.subtract)
                    nc.vector.tensor_tensor(t1, S1, S2, ALU.add)
                    nc.vector.tensor_tensor(t2, g(6), g(7), ALU.add)
                    nc.vector.tensor_tensor(e3, t1, t2, ALU.subtract)
                    nc.vector.tensor_tensor(A, t1, t2, ALU.add)
                Q1 = wp.tile([P, NB, 144], F32, tag="Q1", name="Q1", bufs=1)
                Q2 = wp.tile([P, NB, 144], F32, tag="Q2", name="Q2", bufs=1)
                Q3 = wp.tile([P, NB, 144], F32, tag="Q3", name="Q3", bufs=1)
                DEN = wp.tile([P, NB, 144], F32, tag="DEN", name="DEN", bufs=1)
                RT = wp.tile([P, NB, 144], F32, tag="RT", name="RT", bufs=1)
                PSI = wp.tile([P, NB, 144], F32, tag="PSI", name="PSI", bufs=1)
                PMD = wp.tile([P, 144], F32, tag="PMD", name="PMD", bufs=1)
                nc.gpsimd.tensor_tensor(Q1, e1, e1, ALU.mult)
                nc.gpsimd.tensor_tensor(Q2, e2, e2, ALU.mult)
                nc.gpsimd.tensor_tensor(Q3, e3, e3, ALU.mult)
                nc.scalar.activation(DEN, A, ACTF.Square, scale=0.25,
                                     bias=EPS)
                nc.vector.reciprocal(RT, DEN)
                nc.gpsimd.tensor_add(Q1, Q1, Q2)
                nc.gpsimd.tensor_scalar(Q3, Q3, 0.5, None, ALU.mult)
                nc.gpsimd.tensor_add(Q1, Q1, Q3)
                # psi = Q1 * RT / 6  (Q1 = (q1+q2) + q3/2 = 2*num)
                nc.gpsimd.tensor_tensor(PSI, Q1, RT, ALU.mult)
                nc.gpsimd.tensor_scalar(PSI, PSI, 1.0 / 6, None, ALU.mult)
                for i in range(NB):
                    nc.scalar.activation(
                        PMD, PSI[:, i], ACTF.Identity,
                        accum_out=psi_acc[:, s0 + i:s0 + i + 1])

            prev_stats = None
            for s in range(segs):
                if s % NB == 0:
                    SG3 = wp.tile([P, NB, 8, NWIN * NWIN], BF16, tag="SG3",
                                  name="SG3")
                    E4 = wp.tile([P, NB, 4, NWIN * NWIN], BF16, tag="E4",
                                 name="E4")
                sl = s % NB
                sg_wj = SG3.rearrange("p b g (wi wj) -> p b g wi wj",
                                      wj=NWIN)
                for half in range(2):
                    PS = mmp.tile([P, 6, 256], F32, tag="PS", name="PS")
                    SQA = wp.tile([P, 6, 8, NWIN], BF16, tag="SQA",
                                  name="SQA")
                    for k in range(6):
                        wj = half * 6 + k
                        if wj <= 5:
                            lhsT = T0v[:, s]
                            rhs = WB0v[:, wj]
                        else:
                            lhsT = T1v[:, s]
                            rhs = WB1v[:, wj - 5]
                        nc.tensor.matmul(PS[:, k, 0:192], lhsT, rhs,
                                         start=True, stop=True)
                    pv = PS[:, :, 0:192].rearrange(
                        "p w (g wi c) -> p w g wi c", g=8, c=2)
                    with nc.allow_low_precision(reason="bf16 sigma"):
                        nc.scalar.activation(SQA, pv[:, :, :, :, 0],
                                             ACTF.Square)
                        # out [p, g, wi, wj(6)] <- in0 psum odd (g, wi)
                        # per wj slot, in1 SQA
                        ov_sg = sg_wj[:, sl].rearrange(
                            "p g wi wj -> p (g wi) wj")[
                                :, :, half * 6:half * 6 + 6]
                        in0 = pv[:, :, :, :, 1].rearrange(
                            "p w g wi -> p (g wi) w")
                        in1 = SQA.rearrange("p w g wi -> p (g wi) w")
                        nc.vector.ath_sqadd(ov_sg, in0, in1)
                    if s < 8 or half == 0:
                        emit_sort(1)
                with nc.allow_low_precision(reason="bf16 sigma"):
                    nc.scalar.activation(SG3[:, sl], SG3[:, sl], ACTF.Sqrt,
                                         scale=1.0 / 3)
                if s >= 28 and (s - 28) % 2 == 0 and (s - 28) // 2 < 8:
                    emit_scan_chunk((s - 28) // 2)
                if s % NB == NB - 1:
                    if prev_stats is not None:
                        emit_stats(prev_stats)
                    prev_stats = (SG3, E4, s - NB + 1)
            emit_stats(prev_stats)
            emit_sort(36)
            mm_ctx.__exit__(None, None, None)
            wp_ctx.__exit__(None, None, None)

            # ---- richness = psi_m * entropy, mean over channels ----
            nc.vector.tensor_scalar(
                e_acc, e_acc, -1.0 / (NPIX * LN2), float(math.log2(NPIX)),
                ALU.mult, ALU.add)
            nc.vector.scalar_tensor_tensor(
                rich, psi_acc, 1.0 / (NWIN * NWIN), e_acc,
                ALU.mult, ALU.mult)
            nc.vector.tensor_add(tsum, rich3[:, :, 0], rich3[:, :, 1])
            nc.vector.tensor_add(tsum, tsum, rich3[:, :, 2])
            nc.vector.tensor_scalar(osb, tsum, 1.0 / C, None, ALU.mult)

            # ---- output DMAs ----
            for t in range(T_BLKS):
                b = t // (T_BLKS // B_CORE)
                hp0 = (t % (T_BLKS // B_CORE)) * 4
                nc.sync.dma_start(ov[b, hp0:hp0 + 4], osb[:, t:t + 1])

    nc.compile()
    return nc


@functools.lru_cache(maxsize=4)
def _build_cached(dct_flat: tuple) -> bass.Bass:
    return _build(dct_flat)


def kernel(x, dct_matrix):
    x = np.ascontiguousarray(np.asarray(x, dtype=np.float32))
    D = np.asarray(dct_matrix, dtype=np.float32)
    assert x.shape == (B_FULL, C, H, W), x.shape
    dct_flat = tuple(float(v) for v in D.flatten())
    nc = _build_cached(dct_flat)
    W0, W1 = build_wsh(dct_flat)
    in_maps = [
        {"x": np.ascontiguousarray(x[i * B_CORE:(i + 1) * B_CORE]),
         "w0": W0, "w1": W1}
        for i in range(N_CORES)
    ]
    res = bass_utils.run_bass_kernel_spmd(
        nc, in_maps, core_ids=list(range(N_CORES)))
    out = np.concatenate([r["out"] for r in res.results], axis=0)
    return out.astype(np.float32)


# revision 24
# speedup vs baseline: 1.0221x; 1.0221x over previous
"""Trainium2 Bass kernel for nn_DirectionVarEntropy.

Computes, per 14x14 patch and channel:
  - pixel-value entropy (256-bin histogram of round(x*255))
  - direction variance psi of 3x3-DCT sliding-window directional stds
  - richness = mean_c(psi_m * entropy)  ->  output (B, Hp, Wp)

Sharding: pure data parallel over batch, 2 images per core on 8 cores.

Per-core layout: 2048 spatial patches x 3 channels = 6144 patch-channels,
mapped to [128 partitions x 48 free segments]; seg s = t*3 + c where
t = spatial_patch // 128, partition p = spatial_patch % 128.

Entropy: sort each seg's 196 pixel codes (bitonic merge network, 36
min/max stages in bf16 at the DVE 2x rate), then per-pixel own-bin
counts from run lengths via custom DVE scans; E = log2(196) -
mean_p ln(c_p)/ln 2.  Sort stages are interleaved into the per-seg psi
pipeline as DVE filler.

Psi (new): per direction group g the 3 DCT values' variance is written
with two orthogonal contrasts d1,d2 (linear in the window pixels):
sigma_g^2 * 3 = d1^2 + d2^2.  All 16 contrast planes for the 144
windows of a (seg, wj) column come from ONE PE matmul with the patch
data as the stationary operand ([126pix x 128patch] lhsT) and a
wj-shifted constant weight table as the moving operand ([126 x 192]).
Squares: even contrasts via ACT Square (PSUM->SBUF, fused drain), odd
via a custom DVE op out = Src0^2 + Src1 (one PSUM input allowed).
sigma via ACT Sqrt; S1..S4/A/e-contrasts as strided bf16 adds; psi =
(e1^2/2 + e2^2/2 + e3^2/4) / (3 (A/4 + 1e-8)^2).
"""

import functools
import math

import numpy as np

import concourse.bacc as bacc
import concourse.bass as bass
import concourse.mybir as mybir
from concourse import bass_utils
from concourse.tile import TileContext
from concourse.masks import make_identity

# ---------------- custom DVE ops (registered at import) ----------------
import concourse.dve_ops as dve_ops
from concourse.dve_spec import (Spec, Src0, Src1, C0, C1, Zero, MaxNeg,
                                eq, maxx, select, scan, lower as dve_lower,
                                AluOp, Idx, _has_src1)
from concourse.dve_uop import DveOpSpec
from concourse.bass import BassVectorEngine


def _register(name: str, spec: Spec, subdim: bool = False):
    for op in dve_ops.OPS:
        if op.name == name:
            return op
    row = dve_ops._CUSTOM_DVE_ROW_BASE + len(dve_ops.OPS)
    assert row < 0x20, "custom DVE op rows exhausted"
    shas = {}
    for ver in ("v3", "v4"):
        s = DveOpSpec(name=name, opcode=row, uops=dve_lower(spec, ver=ver),
                      rd1_en=_has_src1(spec))
        shas[ver] = s.sha(ver)
    op = dve_ops.DveOp(name, spec, subdim=subdim, uops_sha=shas)
    dve_ops.OPS.append(op)
    dve_ops._SUB_OPCODE_FOR_NAME[name] = row
    dve_ops.CUSTOM_DVE_SPECS[name] = spec
    return op


def _np_bidx(fill):
    def ref(in0, in1, s0, s1, imm2):
        n = int(np.prod(in0.shape[1:]))
        idx = np.arange(n, dtype=np.float32).reshape((1,) + in0.shape[1:])
        return np.where(in0 == in1, np.float32(fill), idx).astype(np.float32)
    return ref


_BIDX_NEG = _register(
    "ATH_BIDX_NEG",
    Spec(body=select(eq(Src0, Src1), MaxNeg, Idx),
         reference=_np_bidx(-3.4028235e38)))
_BIDX_POS = _register(
    "ATH_BIDX_POS",
    Spec(body=select(eq(Src0, Src1), Zero - MaxNeg, Idx),
         reference=_np_bidx(3.4028235e38)))
_SCAN_MAX = _register(
    "ATH_SCAN_MAX",
    Spec(body=scan(AluOp.MAX, Src0),
         reference=lambda in0, in1, s0, s1, imm2:
         np.maximum.accumulate(
             in0.reshape(in0.shape[0], -1), axis=1).reshape(in0.shape)))
_SCAN_MIN = _register(
    "ATH_SCAN_MIN",
    Spec(body=scan(AluOp.MIN, Src0, init=C0),
         reference=lambda in0, in1, s0, s1, imm2:
         np.minimum.accumulate(
             np.minimum(in0, s0).reshape(in0.shape[0], -1),
             axis=1).reshape(in0.shape)))
_CONV2 = _register(
    "ATH_CONV2",
    Spec(body=Src0 * C0 + Src1 * C1,
         reference=lambda in0, in1, s0, s1, imm2: in0 * s0 + in1 * s1))
_SQADD = _register(
    "ATH_SQADD",
    Spec(body=Src0 * Src0 + Src1,
         reference=lambda in0, in1, s0, s1, imm2: in0 * in0 + in1))


def _v_bidx_neg(self, out, in0, in1):
    return self._custom_dve(_BIDX_NEG, out=out, in0=in0, in1=in1)


def _v_bidx_pos(self, out, in0, in1):
    return self._custom_dve(_BIDX_POS, out=out, in0=in0, in1=in1)


def _v_scan_max(self, out, in0):
    return self._custom_dve(_SCAN_MAX, out=out, in0=in0)


def _v_scan_min(self, out, in0, init):
    return self._custom_dve(_SCAN_MIN, out=out, in0=in0, s0=init)


def _v_conv2(self, out, in0, in1, c0, c1):
    return self._custom_dve(_CONV2, out=out, in0=in0, in1=in1, s0=c0, s1=c1)


def _v_sqadd(self, out, in0, in1):
    return self._custom_dve(_SQADD, out=out, in0=in0, in1=in1)


BassVectorEngine.ath_bidx_neg = _v_bidx_neg
BassVectorEngine.ath_bidx_pos = _v_bidx_pos
BassVectorEngine.ath_scan_max = _v_scan_max
BassVectorEngine.ath_scan_min = _v_scan_min
BassVectorEngine.ath_conv2 = _v_conv2
BassVectorEngine.ath_sqadd = _v_sqadd

# ---------------- problem constants ----------------
P = 128
PH = 14
NWIN = 12          # sliding 3x3 positions per axis
NPIX = PH * PH     # 196
LN2 = 0.6931471805599453
F32 = mybir.dt.float32
BF16 = mybir.dt.bfloat16
ALU = mybir.AluOpType
ACTF = mybir.ActivationFunctionType

B_FULL, C, H, W = 16, 3, 448, 448
N_CORES = 8
B_CORE = B_FULL // N_CORES      # 2
HP = H // PH                    # 32
T_BLKS = B_CORE * HP * HP // P  # 16 t-blocks of 128 spatial patches
SEGS = T_BLKS * C               # 48

SW = 256                        # sort width per seg (padded)
PADV = 320.0                    # pad value > max code 255
SDOM = NPIX + 1                 # scan domain per seg (196 codes + 1 pad)
BIG = 3.0e38

GROUPS = ([[(r, 0), (r, 1), (r, 2)] for r in range(3)]
          + [[(0, c), (1, c), (2, c)] for c in range(3)]
          + [[(0, 0), (1, 1), (2, 2)], [(0, 2), (1, 1), (2, 0)]])
C1V = np.array([1.0, -1.0, 0.0]) / np.sqrt(2.0)
C2V = np.array([1.0, 1.0, -2.0]) / np.sqrt(6.0)


def build_wsh(dct_flat):
    """Shifted contrast weight tables (f32).
    W0 [126, 6*192] for wj 0..5 against XT tile T0 (pix rows 0..125),
    W1 [126, 7*192] for wj 5..11 against T1 (pix rows 70..195).
    Feature index f = g*24 + wi*2 + c2; pixel row = dj*14 + i."""
    D = np.asarray(dct_flat, np.float64).reshape(3, 3)
    W = np.zeros((3, PH, 8, NWIN, 2), np.float64)
    for g, mem in enumerate(GROUPS):
        for c2i, cvec in enumerate((C1V, C2V)):
            for mi, (r, c) in enumerate(mem):
                for di in range(3):
                    for wi in range(NWIN):
                        for dj in range(3):
                            W[dj, wi + di, g, wi, c2i] += (
                                cvec[mi] * D[r, di] * D[c, dj])
    W42 = W.reshape(42, 192)
    W0 = np.zeros((126, 6, 192), np.float64)
    for wj in range(6):
        W0[14 * wj:14 * wj + 42, wj] = W42
    W1 = np.zeros((126, 7, 192), np.float64)
    for wj in range(5, 12):
        base = 14 * (wj - 5)
        W1[base:base + 42, wj - 5] = W42
    return (W0.reshape(126, -1).astype(np.float32),
            W1.reshape(126, -1).astype(np.float32))


def _sort_stages(nc, SRT, Bp):
    """Generator yielding one bitonic stage (2 DVE instrs) per next().
    See baseline docstring: pads at the end are global maxima; trailing
    all-pad merge blocks are skipped and must be pre-set in BOTH buffers."""
    ping = lambda: SRT[:, :, 1:1 + SW]
    pong = lambda: Bp[:, :, :]
    cur_in, cur_out = ping, pong

    def cmpex(lo_out, hi_out, lo_a, lo_b, hi_a, hi_b):
        nc.vector.tensor_tensor(lo_out, lo_a, lo_b, ALU.min)
        nc.vector.tensor_tensor(hi_out, hi_a, hi_b, ALU.max)

    nphase = SW.bit_length() - 1          # 8
    for j in range(nphase):
        m = 2 << j
        h = m // 2
        na = (NPIX + m - 1) // m          # active merge blocks
        nfull = NPIX // m                 # blocks fully below the boundary
        ain, aout = cur_in(), cur_out()
        i4 = ain.rearrange("p s (nb m) -> p s nb m", m=m)[:, :, 0:na]
        o4 = aout.rearrange("p s (nb m) -> p s nb m", m=m)[:, :, 0:na]

        def tri(blo, bhi, kmin, kmax):
            ii = i4[:, :, blo:bhi]
            oo = o4[:, :, blo:bhi]
            if h > 1:
                hr = ii[:, :, :, m - 1:h - 1:-1]
                lr = ii[:, :, :, h - 1::-1]
            else:
                hr = ii[:, :, :, m - 1:m]
                lr = ii[:, :, :, 0:1]
            if kmin > 0:
                nc.vector.tensor_tensor(
                    oo[:, :, :, 0:kmin], ii[:, :, :, 0:kmin],
                    hr[:, :, :, 0:kmin], ALU.min)
            if kmax > 0:
                nc.vector.tensor_tensor(
                    oo[:, :, :, h:h + kmax], ii[:, :, :, h:h + kmax],
                    lr[:, :, :, 0:kmax], ALU.max)

        if nfull > 0:
            tri(0, nfull, h, h)
        if na > nfull:                    # boundary block, partial lanes
            base = nfull * m
            tri(nfull, na, min(h, NPIX - base),
                max(0, NPIX - base - h))
        cur_in, cur_out = cur_out, cur_in
        yield
        d = h // 2
        while d >= 1:
            nc_act = (NPIX + 2 * d - 1) // (2 * d)
            ain, aout = cur_in(), cur_out()
            i4 = ain.rearrange("p s (nb t) -> p s nb t",
                               t=2 * d)[:, :, 0:nc_act]
            o4 = aout.rearrange("p s (nb t) -> p s nb t",
                                t=2 * d)[:, :, 0:nc_act]
            cmpex(o4[:, :, :, 0:d], o4[:, :, :, d:2 * d],
                  i4[:, :, :, 0:d], i4[:, :, :, d:2 * d],
                  i4[:, :, :, 0:d], i4[:, :, :, d:2 * d])
            cur_in, cur_out = cur_out, cur_in
            yield
            d //= 2
    assert cur_in == ping


def _build(dct_flat: tuple) -> bass.Bass:
    nc = bacc.Bacc("TRN2", debug=False, enable_asserts=False)

    x_d = nc.dram_tensor("x", (B_CORE, C, H, W), F32, kind="ExternalInput")
    w0_d = nc.dram_tensor("w0", (126, 6 * 192), F32, kind="ExternalInput")
    w1_d = nc.dram_tensor("w1", (126, 7 * 192), F32, kind="ExternalInput")
    out_d = nc.dram_tensor("out", (B_CORE, HP, HP), F32, kind="ExternalOutput")
    xv = x_d.ap().rearrange("b c (hp i) (wp j) -> b c hp wp i j", i=PH, j=PH)
    ov = out_d.ap()

    segs = SEGS

    with TileContext(nc) as tc:
        with tc.tile_pool(name="persist", bufs=1) as pp:
            lnd = pp.tile([P, NPIX], F32)
            CNT = pp.tile([P, segs, NPIX], BF16)
            psi_acc = pp.tile([P, segs], F32)
            e_acc = pp.tile([P, segs], F32)
            rich = pp.tile([P, segs], F32)
            rich3 = rich.rearrange("p (t c) -> p t c", c=C)
            tsum = pp.tile([P, segs // C], F32)
            osb = pp.tile([P, segs // C], F32)
            ident = pp.tile([P, P], BF16)
            WT0 = pp.tile([126, 6 * 192], F32)
            WT1 = pp.tile([126, 7 * 192], F32)
            WB0 = pp.tile([126, 6 * 192], BF16)
            WB1 = pp.tile([126, 7 * 192], BF16)
            T0 = pp.tile([126, segs * P], BF16)
            T1 = pp.tile([126, segs * P], BF16)
            T0v = T0.rearrange("q (s p) -> q s p", p=P)
            T1v = T1.rearrange("q (s p) -> q s p", p=P)
            EPS = pp.tile([P, 1], F32)
            b23n = pp.tile([P, 1], F32)
            SRT = pp.tile([P, segs, 1 + SW], BF16)

            xq_ctx = tc.tile_pool(name="xq", bufs=1)
            xq = xq_ctx.__enter__()
            X = xq.tile([P, segs, PH, PH], F32)
            Xf = X.rearrange("p s i j -> p (s i j)")
            TMP = xq.tile([P, (segs // 8) * NPIX], F32)

            def issue_seg_dmas(s):
                t, c = s // C, s % C
                b = t // (T_BLKS // B_CORE)
                hp0 = (t % (T_BLKS // B_CORE)) * 4
                engs = (nc.sync, nc.sync, nc.sync, nc.sync)
                for p1 in range(4):
                    engs[p1].dma_start(
                        X[p1 * 32:(p1 + 1) * 32, s],
                        xv[b, c, hp0 + p1],
                    )

            nc.sync.dma_start(WT0, w0_d.ap())
            nc.sync.dma_start(WT1, w1_d.ap())
            make_identity(nc, ident)
            nc.vector.memset(EPS, 1e-8)
            nc.vector.memset(b23n, -float(2 ** 23))
            with nc.allow_low_precision(reason="bf16 matmul weights"):
                nc.scalar.copy(WB0, WT0)
                nc.scalar.copy(WB1, WT1)


            # ---- Xbji (bf16, j-major) + XT via PE transposes ----
            xb_ctx = tc.tile_pool(name="xb", bufs=1)
            xb = xb_ctx.__enter__()
            Xbji = xb.tile([P, segs, PH, PH], BF16)   # [p, s, j, i]
            Xjf = Xbji.rearrange("p s j i -> p s (j i)")

            tp_ctx = tc.tile_pool(name="tp", bufs=2, space="PSUM")
            tpp = tp_ctx.__enter__()
            NS8 = 8
            for s8 in range(segs // NS8):
                g0 = s8 * NS8
                for k in range(NS8):
                    issue_seg_dmas(g0 + k)
                with nc.allow_low_precision(reason="bf16 matmul input"):
                    nc.scalar.copy(
                        Xbji[:, g0:g0 + NS8],
                        X[:, g0:g0 + NS8].rearrange("p s i j -> p s j i"))
                TA = tpp.tile([96, NS8, P], BF16, tag="TA", name="TA")
                TB = tpp.tile([32, NS8, P], BF16, tag="TB", name="TB")
                TC = tpp.tile([96, NS8, P], BF16, tag="TC", name="TC")
                TD = tpp.tile([32, NS8, P], BF16, tag="TD", name="TD")
                for k in range(NS8):
                    s = s8 * NS8 + k
                    nc.tensor.transpose(TA[:, k], Xjf[:, s, 0:96], ident)
                    nc.tensor.transpose(TB[0:30, k], Xjf[:, s, 96:126],
                                        ident)
                    nc.tensor.transpose(TC[:, k], Xjf[:, s, 70:166], ident)
                    nc.tensor.transpose(TD[0:30, k], Xjf[:, s, 166:196],
                                        ident)
                sl = slice(s8 * NS8 * P, (s8 * NS8 + NS8) * P)
                with nc.allow_low_precision(reason="bf16 XT"):
                    nc.vector.tensor_copy(
                        T0[0:96, sl], TA.rearrange("q k p -> q (k p)"))
                    nc.scalar.copy(
                        T0[96:126, sl],
                        TB[0:30].rearrange("q k p -> q (k p)"))
                    nc.vector.tensor_copy(
                        T1[0:96, sl], TC.rearrange("q k p -> q (k p)"))
                    nc.scalar.copy(
                        T1[96:126, sl],
                        TD[0:30].rearrange("q k p -> q (k p)"))
            tp_ctx.__exit__(None, None, None)
            xb_ctx.__exit__(None, None, None)

            # ---- quantize codes into SRT (ACT + gpsimd, via 2^23 RNE) ----
            TWO23 = float(2 ** 23)
            qch = (segs // 8) * NPIX
            TMP3 = TMP.rearrange("p (s k) -> p s k", k=NPIX)
            spq = segs // 8
            for q in range(8):
                nc.gpsimd.tensor_scalar(
                    TMP, Xf[:, q * qch:(q + 1) * qch], 255.0, TWO23,
                    ALU.mult, ALU.add)
                nc.scalar.activation(
                    SRT[:, q * spq:(q + 1) * spq, 1:1 + NPIX], TMP3,
                    ACTF.Identity, bias=b23n)
            nc.vector.memset(SRT[:, :, 0:1], -1.0)
            nc.vector.memset(SRT[:, :, 1 + NPIX:1 + SW], PADV)
            xq_ctx.__exit__(None, None, None)

            # ============ psi pipeline with interleaved sort ============
            wp_ctx = tc.tile_pool(name="work", bufs=2)
            wp = wp_ctx.__enter__()
            mm_ctx = tc.tile_pool(name="mm", bufs=2, space="PSUM")
            mmp = mm_ctx.__enter__()
            Bp = wp.tile([P, segs, SW], BF16, tag="BP", name="BP", bufs=1)
            nc.gpsimd.memset(Bp[:, :, NPIX:SW], PADV)
            sorter = _sort_stages(nc, SRT, Bp)
            sort_left = 36

            def emit_sort(k):
                nonlocal sort_left
                for _ in range(min(k, sort_left)):
                    next(sorter)
                    sort_left -= 1

            cs = segs // 8

            def emit_scan_chunk(h):
                s0c = h * cs
                src0 = SRT[:, s0c:s0c + cs, 1:1 + SDOM]
                src1 = SRT[:, s0c:s0c + cs, 0:SDOM]
                SC = wp.tile([P, cs * SDOM], F32, tag="SCS", name="SCS",
                             bufs=2)
                MC = wp.tile([P, cs * SDOM], F32, tag="SCM", name="SCM",
                             bufs=1)
                NX = wp.tile([P, cs * SDOM], F32, tag="SCN", name="SCN",
                             bufs=1)
                nc.vector.ath_bidx_neg(MC, src0, src1)
                nc.vector.ath_scan_max(SC, MC)
                nc.vector.ath_bidx_pos(MC, src0, src1)
                nmax = cs * SDOM
                nc.vector.ath_scan_min(NX[:, nmax - 1::-1],
                                       MC[:, nmax - 1::-1], BIG)
                Svc = SC.rearrange("p (s k) -> p s k", k=SDOM)
                NXv = NX.rearrange("p (s k) -> p s k", k=SDOM)
                with nc.allow_low_precision(reason="counts <= 196 exact"):
                    nc.gpsimd.tensor_tensor(
                        CNT[:, s0c:s0c + cs], NXv[:, :, 1:1 + NPIX],
                        Svc[:, :, 0:NPIX], ALU.subtract)
                for i in range(cs):
                    nc.scalar.activation(
                        lnd, CNT[:, s0c + i], ACTF.Ln,
                        accum_out=e_acc[:, s0c + i:s0c + i + 1])

            WB0v = WB0.rearrange("q (w f) -> q w f", f=192)
            WB1v = WB1.rearrange("q (w f) -> q w f", f=192)
            NB = 4   # segs per stats batch

            def emit_stats(sb):
                """S/A/e + psi for a 4-seg batch whose sigma is in SG3."""
                SG3, E4, s0 = sb
                # views [p, b, g, w] with w = 144
                g = lambda i: SG3[:, :, i]
                e1, e2, e3, A = (E4[:, :, 0], E4[:, :, 1], E4[:, :, 2],
                                 E4[:, :, 3])
                S1, S2, t1, t2 = g(0), g(3), g(1), g(6)
                with nc.allow_low_precision(reason="bf16 stats"):
                    nc.vector.ath_conv2(S1, g(0), g(1), 1 / 3, 1 / 3)
                    nc.vector.scalar_tensor_tensor(
                        S1, g(2), 1 / 3, S1, ALU.mult, ALU.add)
                    nc.vector.ath_conv2(S2, g(3), g(4), 1 / 3, 1 / 3)
                    nc.vector.scalar_tensor_tensor(
                        S2, g(5), 1 / 3, S2, ALU.mult, ALU.add)
                    nc.vector.tensor_tensor(e1, S1, S2, ALU.subtract)
                    nc.vector.tensor_tensor(e2, g(6), g(7), ALU.subtract)
                    nc.vector.tensor_tensor(t1, S1, S2, ALU.add)
                    nc.vector.tensor_tensor(t2, g(6), g(7), ALU.add)
                    nc.vector.tensor_tensor(e3, t1, t2, ALU.subtract)
                    nc.vector.tensor_tensor(A, t1, t2, ALU.add)
                Q1 = wp.tile([P, NB, 144], F32, tag="Q1", name="Q1", bufs=1)
                Q2 = wp.tile([P, NB, 144], F32, tag="Q2", name="Q2", bufs=1)
                Q3 = wp.tile([P, NB, 144], F32, tag="Q3", name="Q3", bufs=1)
                DEN = wp.tile([P, NB, 144], F32, tag="DEN", name="DEN", bufs=1)
                RT = wp.tile([P, NB, 144], F32, tag="RT", name="RT", bufs=1)
                PSI = wp.tile([P, NB, 144], F32, tag="PSI", name="PSI", bufs=1)
                PMD = wp.tile([P, 144], F32, tag="PMD", name="PMD", bufs=1)
                nc.gpsimd.tensor_tensor(Q1, e1, e1, ALU.mult)
                nc.gpsimd.tensor_tensor(Q2, e2, e2, ALU.mult)
                nc.gpsimd.tensor_tensor(Q3, e3, e3, ALU.mult)
                nc.scalar.activation(DEN, A, ACTF.Square, scale=0.25,
                                     bias=EPS)
                nc.vector.reciprocal(RT, DEN)
                nc.gpsimd.tensor_add(Q1, Q1, Q2)
                nc.gpsimd.tensor_scalar(Q3, Q3, 0.5, None, ALU.mult)
                nc.gpsimd.tensor_add(Q1, Q1, Q3)
                # psi = Q1 * RT / 6  (Q1 = (q1+q2) + q3/2 = 2*num)
                nc.gpsimd.tensor_tensor(PSI, Q1, RT, ALU.mult)
                nc.gpsimd.tensor_scalar(PSI, PSI, 1.0 / 6, None, ALU.mult)
                for i in range(NB):
                    nc.scalar.activation(
                        PMD, PSI[:, i], ACTF.Identity,
                        accum_out=psi_acc[:, s0 + i:s0 + i + 1])

            prev_stats = None
            for s in range(segs):
                if s % NB == 0:
                    SG3 = wp.tile([P, NB, 8, NWIN * NWIN], BF16, tag="SG3",
                                  name="SG3")
                    E4 = wp.tile([P, NB, 4, NWIN * NWIN], BF16, tag="E4",
                                 name="E4", bufs=1)
                sl = s % NB
                sg_wj = SG3.rearrange("p b g (wi wj) -> p b g wi wj",
                                      wj=NWIN)
                for half in range(2):
                    PS = mmp.tile([P, 6, 256], F32, tag="PS", name="PS")
                    SQA = wp.tile([P, 6, 8, NWIN], BF16, tag="SQA",
                                  name="SQA")
                    for k in range(6):
                        wj = half * 6 + k
                        if wj <= 5:
                            lhsT = T0v[:, s]
                            rhs = WB0v[:, wj]
                        else:
                            lhsT = T1v[:, s]
                            rhs = WB1v[:, wj - 5]
                        nc.tensor.matmul(PS[:, k, 0:192], lhsT, rhs,
                                         start=True, stop=True)
                    pv = PS[:, :, 0:192].rearrange(
                        "p w (g wi c) -> p w g wi c", g=8, c=2)
                    with nc.allow_low_precision(reason="bf16 sigma"):
                        nc.scalar.activation(SQA, pv[:, :, :, :, 0],
                                             ACTF.Square)
                        # out [p, g, wi, wj(6)] <- in0 psum odd (g, wi)
                        # per wj slot, in1 SQA
                        ov_sg = sg_wj[:, sl].rearrange(
                            "p g wi wj -> p (g wi) wj")[
                                :, :, half * 6:half * 6 + 6]
                        in0 = pv[:, :, :, :, 1].rearrange(
                            "p w g wi -> p (g wi) w")
                        in1 = SQA.rearrange("p w g wi -> p (g wi) w")
                        nc.vector.ath_sqadd(ov_sg, in0, in1)
                    if s < 8 or half == 0:
                        emit_sort(1)
                with nc.allow_low_precision(reason="bf16 sigma"):
                    nc.scalar.activation(SG3[:, sl], SG3[:, sl], ACTF.Sqrt,
                                         scale=1.0 / 3)
                if s >= 32 and (s - 32) % 2 == 0 and (s - 32) // 2 < 8:
                    emit_scan_chunk((s - 32) // 2)
                if s % NB == NB - 1:
                    if prev_stats is not None:
                        emit_stats(prev_stats)
                    prev_stats = (SG3, E4, s - NB + 1)
            # ---- richness = psi_m * entropy, per t-block ----
            def emit_rich(t0, t1):
                sl = slice(t0 * C, t1 * C)
                nc.vector.tensor_scalar(
                    e_acc[:, sl], e_acc[:, sl], -1.0 / (NPIX * LN2),
                    float(math.log2(NPIX)), ALU.mult, ALU.add)
                nc.vector.scalar_tensor_tensor(
                    rich[:, sl], psi_acc[:, sl], 1.0 / (NWIN * NWIN),
                    e_acc[:, sl], ALU.mult, ALU.mult)
                nc.vector.tensor_add(tsum[:, t0:t1], rich3[:, t0:t1, 0],
                                     rich3[:, t0:t1, 1])
                nc.vector.tensor_add(tsum[:, t0:t1], tsum[:, t0:t1],
                                     rich3[:, t0:t1, 2])
                nc.vector.tensor_scalar(osb[:, t0:t1], tsum[:, t0:t1],
                                        1.0 / C, None, ALU.mult)
                for t in range(t0, t1):
                    b = t // (T_BLKS // B_CORE)
                    hp0 = (t % (T_BLKS // B_CORE)) * 4
                    nc.sync.dma_start(ov[b, hp0:hp0 + 4], osb[:, t:t + 1])

            emit_rich(0, 13)
            emit_stats(prev_stats)
            emit_sort(36)
            emit_rich(13, T_BLKS)
            mm_ctx.__exit__(None, None, None)
            wp_ctx.__exit__(None, None, None)

    nc.compile()
    return nc


@functools.lru_cache(maxsize=4)
def _build_cached(dct_flat: tuple) -> bass.Bass:
    return _build(dct_flat)


def kernel(x, dct_matrix):
    x = np.ascontiguousarray(np.asarray(x, dtype=np.float32))
    D = np.asarray(dct_matrix, dtype=np.float32)
    assert x.shape == (B_FULL, C, H, W), x.shape
    dct_flat = tuple(float(v) for v in D.flatten())
    nc = _build_cached(dct_flat)
    W0, W1 = build_wsh(dct_flat)
    in_maps = [
        {"x": np.ascontiguousarray(x[i * B_CORE:(i + 1) * B_CORE]),
         "w0": W0, "w1": W1}
        for i in range(N_CORES)
    ]
    res = bass_utils.run_bass_kernel_spmd(
        nc, in_maps, core_ids=list(range(N_CORES)))
    out = np.concatenate([r["out"] for r in res.results], axis=0)
    return out.astype(np.float32)
